# revision 4
# baseline (speedup 1.0000x reference)
"""Embedding lookup (one_hot(x) @ W.T + b) as a Bass/Trainium2 kernel.

Problem shapes (hardcoded; see harness contract):
    x: [16, 8192] int   (class ids < 4096)
    W: [512, 4096] f32  (nn.Linear weight; we gather rows of W.T)
    b: [512] f32
    out: [16, 8192, 512] f32 = take(W.T, x, axis=0) + b

Strategy: data-parallel over the 8 NeuronCores; each core handles 16384
tokens.  The whole table (8 MB) lives in SBUF emb-major (4 stripes of
[128, 4096] f32 = rows of W) with the bias pre-added once at setup, so
the steady-state loop never reads the table from HBM: per 2048-token
chunk,

    gpsimd.ap_gather : 4x SBUF->SBUF emb-major gather  [128, 4, 2048]
    tensor.transpose : 64x [128,128] f32 PE transposes -> PSUM,
                       yielding token-major [128, 512] tiles
    scalar.copy      : PSUM -> SBUF staging
    sync.dma_start   : staging -> HBM (the only bulk HBM traffic)

Engines are decoupled with per-slot counting semaphores (values stay
small enough for looped-rep benching).
"""

import numpy as np

import concourse.bacc as bacc
import concourse.mybir as mybir
from concourse.bass_utils import run_bass_kernel_spmd
from concourse.library_config import ap_gather as ap_gather_lib

N_CORES = 8
NCLS = 4096          # table rows (classes)
EMB = 512            # embedding dim
NS = EMB // 128      # 4 emb stripes of 128 partitions
TOK = 16384          # tokens per core (131072 / 8)
CH = 2048            # tokens per gather chunk
NCH = TOK // CH      # 8 chunks per rep
NSUB = CH // 128     # 16 transpose sub-chunks per chunk
GD = 4               # sub-chunks per drain DMA (512 tokens, 1 MB)
NGRP = TOK // (128 * GD)   # 32 drain groups per rep
NGT = 2              # gather tiles in flight
NPS = 4              # PSUM banks in the ring
NST = 2              # staging tiles in flight

TRACE = False        # set by test.py to capture an NTFF profile
LAST_RESULTS = None  # BassKernelResults from the most recent run

_NCS = {}


def _build_nc(reps=1):
    nc = bacc.Bacc("TRN2", debug=False)
    f32 = mybir.dt.float32

    w = nc.dram_tensor("w", [EMB, NCLS], f32, kind="ExternalInput")
    bias2 = nc.dram_tensor("bias2", [128, NS], f32, kind="ExternalInput")
    ident = nc.dram_tensor("ident", [128, 128], f32, kind="ExternalInput")
    idx = nc.dram_tensor("idx", [128, TOK // 16], mybir.dt.int16,
                         kind="ExternalInput")
    out = nc.dram_tensor("out", [TOK, EMB], f32, kind="ExternalOutput")
    _build_body(nc, w, bias2, ident, idx, out, reps)
    nc.compile()
    return nc


def _build_body(nc, w, bias2, ident, idx, out, reps=1):
    f32 = mybir.dt.float32
    # out rows grouped for drain DMAs: row = g*512 + c*128 + p
    out_v = out[:].rearrange("(g c p) e -> g p c e", p=128, c=GD)
    w_v = w[:].rearrange("(s p) c -> s p c", p=128)

    from contextlib import ExitStack

    with (
        nc.sbuf_tensor("w_sb", [128, NS, NCLS], f32) as w_sb,
        nc.sbuf_tensor("b_sb", [128, NS], f32) as b_sb,
        nc.sbuf_tensor("id_sb", [128, 128], f32) as id_sb,
        nc.sbuf_tensor("idx_sb", [128, TOK // 16], mybir.dt.int16) as idx_sb,
        nc.semaphore("io_sem") as io_sem,
        nc.semaphore("bias_done") as bias_done,
        nc.semaphore("g_sem") as g_sem,
        ExitStack() as stack,
        nc.Block() as block,
    ):
        gtiles = [
            stack.enter_context(nc.sbuf_tensor(f"gt{n}", [128, NS, CH], f32))
            for n in range(NGT)
        ]
        stages = [
            stack.enter_context(nc.sbuf_tensor(f"st{n}", [128, GD, EMB], f32))
            for n in range(NST)
        ]
        psums = [
            stack.enter_context(nc.psum_tensor(f"ps{n}", [128, EMB], f32))
            for n in range(NPS)
        ]
        t_sems = [stack.enter_context(nc.semaphore(f"t{q}")) for q in range(NPS)]
        c_sems = [stack.enter_context(nc.semaphore(f"c{q}")) for q in range(NPS)]
        w_sems = [stack.enter_context(nc.semaphore(f"w{q}")) for q in range(NST)]

        n_chunks = reps * NCH

        @block.sync
        def _(sy):
            # setup loads: 4 table stripes + bias + identity + indices
            for s in range(NS):
                sy.dma_start(w_sb[:, s, :], w_v[s]).then_inc(io_sem, 16)
            sy.dma_start(b_sb[:], bias2[:]).then_inc(io_sem, 16)
            sy.dma_start(id_sb[:], ident[:]).then_inc(io_sem, 16)
            sy.dma_start(idx_sb[:], idx[:]).then_inc(io_sem, 16)

            # drain loop
            for G in range(reps * NGRP):
                for q in range(NPS):
                    sy.wait_ge(c_sems[q], G + 1)
                sy.dma_start(
                    out_v[G % NGRP], stages[G % NST][:]
                ).then_inc(w_sems[G % NST], 16)
            for n in range(NST):
                sy.wait_ge(w_sems[n], 16 * (reps * NGRP // NST))

        @block.vector
        def _(vec):
            vec.wait_ge(io_sem, 112)
            vec.tensor_add(
                w_sb[:],
                w_sb[:],
                b_sb[:, :, None].to_broadcast([128, NS, NCLS]),
            ).then_inc(bias_done, 1)

        @block.gpsimd
        def _(gp):
            gp.load_library(ap_gather_lib)
            gp.wait_ge(bias_done, 1)
            for K in range(n_chunks):
                if K >= NGT:
                    # gather tile reuse: chunk K-NGT fully transposed
                    for q in range(NPS):
                        gp.wait_ge(t_sems[q], (NSUB // NPS) * (K - NGT + 1))
                ic = K % NCH
                for s in range(NS):
                    gp.ap_gather(
                        gtiles[K % NGT][:, s, :],
                        w_sb[:, s, :],
                        idx_sb[:, ic * (CH // 16):(ic + 1) * (CH // 16)],
                        channels=128,
                        num_elems=NCLS,
                        d=1,
                        num_idxs=CH,
                    ).then_inc(g_sem, 1)

        @block.tensor
        def _(pe):
            for K in range(n_chunks):
                for i in range(NSUB):
                    J = K * NSUB + i
                    if J >= NPS:
                        pe.wait_ge(c_sems[J % NPS], J // NPS)
                    for s in range(NS):
                        if i == 0:
                            pe.wait_ge(g_sem, NS * K + s + 1)
                        ins = pe.transpose(
                            psums[J % NPS][:, s * 128:(s + 1) * 128],
                            gtiles[K % NGT][:, s, i * 128:(i + 1) * 128],
                            id_sb[:],
                        )
                        if s == NS - 1:
                            ins.then_inc(t_sems[J % NPS], 1)

        @block.scalar
        def _(sc):
            for J in range(n_chunks * NSUB):
                G = J // GD
                sc.wait_ge(t_sems[J % NPS], J // NPS + 1)
                if J % GD == 0 and G >= NST:
                    sc.wait_ge(w_sems[G % NST], 16 * (G // NST))
                sc.copy(
                    stages[G % NST][:, J % GD, :], psums[J % NPS][:]
                ).then_inc(c_sems[J % NPS], 1)


def _get_nc(reps=1):
    if reps not in _NCS:
        _NCS[reps] = _build_nc(reps)
    return _NCS[reps]


def _make_idx_input(xs):
    """Wrap a core's token->class array [TOK] into the int16 SBUF index
    layout: idx_sb[16g + (t % 16), t // 16] = xs[t] for each of the 8
    gpsimd core groups g (replicated)."""
    xs = xs.astype(np.int16)
    wr = xs.reshape(TOK // 16, 16).T          # [16, TOK//16]
    wr = np.tile(wr, (8, 1))                  # [128, TOK//16]
    return np.ascontiguousarray(wr)


def kernel(x, W, b, _reps=1):
    global LAST_RESULTS
    x = np.asarray(x)
    W = np.asarray(W, dtype=np.float32)
    b = np.asarray(b, dtype=np.float32)
    batch, seq = x.shape

    xf = x.reshape(-1)
    w_in = np.ascontiguousarray(W)                      # [512, 4096]
    bias2 = np.ascontiguousarray(b.reshape(NS, 128).T)  # [128, 4]
    ident = np.eye(128, dtype=np.float32)

    per = xf.shape[0] // N_CORES
    assert per == TOK, (xf.shape, TOK)
    in_maps = [
        {
            "w": w_in,
            "bias2": bias2,
            "ident": ident,
            "idx": _make_idx_input(xf[c * per:(c + 1) * per]),
        }
        for c in range(N_CORES)
    ]

    nc = _get_nc(_reps)
    res = run_bass_kernel_spmd(
        nc, in_maps, core_ids=list(range(N_CORES)), trace=TRACE,
    )
    LAST_RESULTS = res

    out = np.concatenate([r["out"] for r in res.results], axis=0)
    return out.reshape(batch, seq, EMB)
